# revision 14
# baseline (speedup 1.0000x reference)
"""Trainium2 Bass kernel for explicit multi-head attention.

Problem: x[2, 2048, 1024], Wq/Wk/Wv/Wo[1024, 1024] (+biases), NHEAD=16.
Sharding over 8 NeuronCores: data-parallel on batch (cores 0-3 -> b=0,
cores 4-7 -> b=1), tensor-parallel over heads (4 heads per core).  Each
core computes its 4 heads' attention plus the partial out-projection
(ctx_local @ Wo[rows_local]); partials are summed on the host, which is
mathematically the all-reduce the sharding hint asks for.

v2 design notes (on top of the v1 transposed-attention layout):
 - Attention starts as early as possible: xT is DMA'd in (k, Lq-chunk)
   pieces, k/q projections for pair-0 chunk-0 are emitted first, and all
   remaining projections (v, later k/q chunks, pair 1, out-proj tiles)
   are drip-fed into the ACT-paced attention loop from a filler queue.
 - PV matmuls run in fp8 DoubleRow perf mode: exp-probs (et) and the
   augmented v tiles are fp8e4; two adjacent key-tiles contract in one
   pass (2x PE throughput).  Softmax weights are quantized but the
   denominator sums the same quantized values, so softmax still sums to
   exactly 1 and the accuracy cost is tiny.
 - Softmax denominators: ctx psum is copied raw to SBUF immediately
   (frees the psum bank in ~1.5us), then reciprocal_approx_fast (~5x
   faster than DVE reciprocal), a partition-broadcast DMA, and two DVE
   multiplies produce the normalized ctxT in bf16.
 - All bias adds ride DVE evacuations (tensor-tensor adds against
   partition-broadcast bias tiles) instead of K=1 rank-1 matmuls.
 - Dummy warm-up matmuls on the first xT piece raise the PE clock gate
   while the input DMAs land.
"""

import os
import sys
from collections import deque

import numpy as np

for _p in ("/opt/trn_rl_repo", "/root/.axon_site/_ro/trn_rl_repo"):
    if os.path.isdir(_p) and _p not in sys.path:
        sys.path.append(_p)

import concourse.bass as bass
import concourse.mybir as mybir
import concourse.tile as tile
from concourse import bacc
from concourse.bass_utils import run_bass_kernel_spmd

# ---------------------------------------------------------------------------
# ACT table-set pinning: this kernel's only activations are Identity and Exp,
# both present in the single "natural_log_exp_and_others" set.  The stock
# per-function chooser maps Exp to "exp_and_others", which alternates
# ACT_TABLE_LOADs (~2.6us each).  Restrict the function->set map so every
# activation resolves to the one set and exactly one table load is emitted.
import concourse.hw_specs as _hw_specs

_orig_get_activation_tables = _hw_specs.get_activation_tables


def _pinned_activation_tables(module_arch):
    t = _orig_get_activation_tables(module_arch)
    pin = {
        mybir.ActivationFunctionType.Exp,
        mybir.ActivationFunctionType.Ln,
        mybir.ActivationFunctionType.Identity,
        mybir.ActivationFunctionType.Copy,
    }
    out = {}
    for name, fns in t.items():
        if name == "natural_log_exp_and_others":
            out[name] = set(fns)
        else:
            out[name] = set(fns) - pin
    return out


bacc.get_activation_tables = _pinned_activation_tables

B = 2
L = 2048
D_MODEL = 1024
NHEAD = 16
D_HEAD = 64
SCALE = 1.0 / float(np.sqrt(D_HEAD))
N_CORES = 8
TP = 4                      # tensor-parallel group size (heads split)
HEADS_PER_CORE = NHEAD // TP          # 4
D_LOCAL = HEADS_PER_CORE * D_HEAD     # 256
N_PAIRS = HEADS_PER_CORE // 2         # 2 head-pairs per core
KT = D_MODEL // 128                   # 8 contraction tiles for projections
LT = L // 128                         # 16 L tiles
NJP = LT // 2                         # 8 key-tile pairs (fp8 DoubleRow)
NCH = L // 512                        # 4 Lq chunks of 512

F32 = mybir.dt.float32
BF16 = mybir.dt.bfloat16
# KERNEL_PV8=0 falls back to bf16 PV matmuls (no fp8 DoubleRow) for debug
PV8 = os.environ.get("KERNEL_PV8", "1") == "1"
FP8 = mybir.dt.float8e4 if PV8 else BF16
PROJ_DT = BF16   # x / Wq / Wk / Wv streamed through the PE
ATT_DT = BF16    # qT / kT score operands
CTX_DT = BF16    # normalized ctxT (out-proj stationary) and Wo

# v_aug stationary layout, one [128, 2, VAUG_W] fp8 window pair per
# (head-pair, key-tile-pair); dim1 indexes the two key-tiles of the
# DoubleRow contraction.  Window offsets/strides are 16B-aligned to satisfy
# the dual-fp8 LDWEIGHTS ISA restrictions:
#   cols 0:64    = v(even head)          -> even window = cols 0:128
#   col  64      = ones (even denom -> ctx_e partition 64)
#   cols 65:144  = zeros, except
#   col  112     = ones (odd denom; odd window = cols 80:208, so col 112
#                  lands on ctx_o partition 32)
#   cols 144:208 = v(odd head)           -> ctx_o partitions 64:128
VAUG_W = 208
ODD_OFF = 80


def _to_bf16(x):
    import ml_dtypes

    return np.ascontiguousarray(np.asarray(x, np.float32).astype(ml_dtypes.bfloat16))


def build_kernel():
    nc = bacc.Bacc("TRN2", target_bir_lowering=False, debug=False)

    xT = nc.dram_tensor("xT", [D_MODEL, L], PROJ_DT, kind="ExternalInput").ap()
    wq = nc.dram_tensor("wq", [D_MODEL, D_LOCAL], PROJ_DT, kind="ExternalInput").ap()
    wk = nc.dram_tensor("wk", [D_MODEL, D_LOCAL], PROJ_DT, kind="ExternalInput").ap()
    wv = nc.dram_tensor("wv", [D_MODEL, D_LOCAL], PROJ_DT, kind="ExternalInput").ap()
    wo = nc.dram_tensor("wo", [D_LOCAL, D_MODEL], CTX_DT, kind="ExternalInput").ap()
    bq = nc.dram_tensor("bq", [D_LOCAL], F32, kind="ExternalInput").ap()
    bk = nc.dram_tensor("bk", [D_LOCAL], F32, kind="ExternalInput").ap()
    bv = nc.dram_tensor("bv", [D_LOCAL], F32, kind="ExternalInput").ap()
    bo = nc.dram_tensor("bo", [D_MODEL], F32, kind="ExternalInput").ap()
    out_p = nc.dram_tensor("out_p", [L, D_MODEL], F32, kind="ExternalOutput").ap()
    DEBUG = os.environ.get("KERNEL_DEBUG", "0") == "1"
    if DEBUG:
        dbg_qT = nc.dram_tensor("dbg_qT", [128, N_PAIRS, L], ATT_DT, kind="ExternalOutput").ap()
        dbg_kT = nc.dram_tensor("dbg_kT", [128, N_PAIRS, L], ATT_DT, kind="ExternalOutput").ap()
        dbg_vaug = nc.dram_tensor("dbg_vaug", [128, N_PAIRS, NJP, 2, VAUG_W], FP8, kind="ExternalOutput").ap()
        dbg_raw = nc.dram_tensor("dbg_raw", [128, 2, 512], F32, kind="ExternalOutput").ap()
        dbg_ctxT = nc.dram_tensor("dbg_ctxT", [128, N_PAIRS, L], CTX_DT, kind="ExternalOutput").ap()
        dbg_xT = nc.dram_tensor("dbg_xT", [128, KT, L], PROJ_DT, kind="ExternalOutput").ap()
        dbg_bc = nc.dram_tensor("dbg_bc", [128, D_LOCAL + D_MODEL], F32, kind="ExternalOutput").ap()

    Exp = mybir.ActivationFunctionType.Exp
    DR = mybir.MatmulPerfMode.DoubleRow

    with tile.TileContext(nc) as tc:
        with (
            tc.tile_pool(name="persist", bufs=1) as persist,
            tc.tile_pool(name="exp_pool", bufs=3) as exp_pool,
            tc.tile_pool(name="raw_pool", bufs=2) as raw_pool,
            tc.tile_pool(name="recip_pool", bufs=2) as recip_pool,
            tc.tile_pool(name="bcs_pool", bufs=2) as bcs_pool,
            tc.tile_pool(name="out_pool", bufs=4) as out_pool,
            tc.tile_pool(name="ps_st", bufs=2, space="PSUM") as ps_st,
            tc.tile_pool(name="ps_acc", bufs=2, space="PSUM") as ps_acc,
            tc.tile_pool(name="ps_work", bufs=2, space="PSUM") as ps_work,
        ):
            # ---- input DMAs, in dependency-priority order ----
            # sync queue: first xT piece (warm-up dep), wk+wq (pair-0 c0
            # projections), rest of xT chunk 0, wv, then the remaining xT
            # chunks.  scalar queue: small/late tensors.
            xT_sb = persist.tile([128, KT, L], PROJ_DT)

            def dma_xT(k, c):
                nc.sync.dma_start(
                    xT_sb[:, k, c * 512 : (c + 1) * 512],
                    xT[k * 128 : (k + 1) * 128, c * 512 : (c + 1) * 512],
                )

            dma_xT(0, 0)
            wk_sb = persist.tile([128, KT, D_LOCAL], PROJ_DT)
            nc.sync.dma_start(wk_sb[:], wk.rearrange("(k p) n -> p k n", p=128))
            wq_sb = persist.tile([128, KT, D_LOCAL], PROJ_DT)
            nc.sync.dma_start(wq_sb[:], wq.rearrange("(k p) n -> p k n", p=128))
            for k in range(1, KT):
                dma_xT(k, 0)
            wv_sb = persist.tile([128, KT, D_LOCAL], PROJ_DT)
            nc.sync.dma_start(wv_sb[:], wv.rearrange("(k p) n -> p k n", p=128))
            for c in range(1, NCH):
                for k in range(KT):
                    dma_xT(k, c)

            bq_sb = persist.tile([128, D_LOCAL // 128], F32)
            nc.scalar.dma_start(bq_sb[:], bq.rearrange("(m p) -> p m", p=128))
            bk_sb = persist.tile([128, D_LOCAL // 128], F32)
            nc.scalar.dma_start(bk_sb[:], bk.rearrange("(m p) -> p m", p=128))
            bv_ld = persist.tile([1, D_LOCAL], F32)
            nc.scalar.dma_start(bv_ld[:], bv.rearrange("(o n) -> o n", o=1))
            bo_ld = persist.tile([1, D_MODEL], F32)
            nc.scalar.dma_start(bo_ld[:], bo.rearrange("(o n) -> o n", o=1))
            wo_sb = persist.tile([128, N_PAIRS, D_MODEL], CTX_DT)
            nc.scalar.dma_start(wo_sb[:], wo.rearrange("(k p) n -> p k n", p=128))
            # partition-broadcast bias tiles (free-dim biases for v/out)
            bv_bc = persist.tile([128, D_LOCAL], F32)
            nc.scalar.dma_start(
                bv_bc[:], bv_ld[0:1, :].unsqueeze(1).broadcast_to([1, 128, D_LOCAL])
            )
            bo_bc = persist.tile([128, D_MODEL], F32)
            nc.scalar.dma_start(
                bo_bc[:], bo_ld[0:1, :].unsqueeze(1).broadcast_to([1, 128, D_MODEL])
            )

            qT_sb = persist.tile([128, N_PAIRS, L], ATT_DT)
            kT_sb = persist.tile([128, N_PAIRS, L], ATT_DT)
            vaug = persist.tile([128, N_PAIRS, NJP, 2, VAUG_W], FP8)
            ctxT_sb = persist.tile([128, N_PAIRS, L], CTX_DT)

            nc.vector.memset(vaug[:, :, :, :, 64:144], 0.0)
            nc.vector.memset(vaug[:, :, :, :, 64:65], 1.0)
            nc.vector.memset(vaug[:, :, :, :, 112:113], 1.0)

            # ---- PE warm-up: dummy matmuls on the first xT piece while the
            # input DMAs land, so the clock-gate ramp starts immediately ----
            warm = ps_work.tile([128, 512], F32, tag="work")
            for i in range(24):
                nc.tensor.matmul(
                    warm[:, 0:128],
                    xT_sb[0:1, 0, 0:128],
                    xT_sb[0:1, 0, 0:128],
                    start=(i == 0),
                    stop=(i == 23),
                )

            # ---- filler jobs: small emission units dripped into the
            # attention loop to keep the PE dense under the ACT pacer ----
            class QKJob:
                """q or k projection for (tensor t, head-pair m, chunk c):
                4 matmul units (2 chained matmuls each) + 1 bias unit."""

                def __init__(self, t, m, c):
                    self.t, self.m, self.c = t, m, c
                    self.ps = None

                def units(self):
                    w_sb, b_sb, dst = (
                        (wq_sb, bq_sb, qT_sb),
                        (wk_sb, bk_sb, kT_sb),
                    )[self.t]
                    m, c = self.m, self.c

                    def mm(kk):
                        def emit():
                            if kk == 0:
                                self.ps = ps_work.tile([128, 512], F32, tag="work")
                            for k in (2 * kk, 2 * kk + 1):
                                nc.tensor.matmul(
                                    self.ps[:],
                                    w_sb[:, k, m * 128 : (m + 1) * 128],
                                    xT_sb[:, k, c * 512 : (c + 1) * 512],
                                    start=(k == 0),
                                    stop=(k == KT - 1),
                                )

                        return emit

                    def evac():
                        nc.vector.tensor_scalar_add(
                            dst[:, m, c * 512 : (c + 1) * 512],
                            self.ps[:],
                            b_sb[:, m : m + 1],
                        )

                    return [mm(kk) for kk in range(4)] + [evac]

            class VJob:
                """v projection for one key-tile lt: 4 matmul units (2 chained
                matmuls each, single accumulation group -- PSUM start zeroing
                is bank-granular so groups must not share a bank) + 1 evac."""

                def __init__(self, lt):
                    self.lt = lt
                    self.ps = None

                def units(self):
                    lt = self.lt

                    def mm(kk):
                        def emit():
                            if kk == 0:
                                # full-bank tile (cols 256:512 unused) so every
                                # ps_work tenant has an identical footprint
                                self.ps = ps_work.tile([128, 512], F32, tag="work")
                            for k in (2 * kk, 2 * kk + 1):
                                nc.tensor.matmul(
                                    self.ps[:, 0:256],
                                    xT_sb[:, k, lt * 128 : (lt + 1) * 128],
                                    wv_sb[:, k, :],
                                    start=(k == 0),
                                    stop=(k == KT - 1),
                                )

                        return emit

                    def evac():
                        # ps cols: [p0_e p0_o p1_e p1_o]; add bv and scatter
                        # into the fp8 vaug windows.
                        ps_r = self.ps[:, 0:256].rearrange(
                            "p (t q n) -> p t q n", t=2, q=2, n=64
                        )
                        bv_r = bv_bc[:].rearrange(
                            "p (t q n) -> p t q n", t=2, q=2, n=64
                        )
                        jp, sub = divmod(lt, 2)
                        for par, dcol in ((0, 0), (1, 144)):
                            nc.vector.tensor_add(
                                vaug[:, :, jp, sub, dcol : dcol + 64],
                                ps_r[:, :, par, :],
                                bv_r[:, :, par, :],
                            )

                    return [mm(kk) for kk in range(4)] + [evac]

            class OutJob:
                """one [128, 512] tile of out[L, D]: 1 matmul unit (2 chained
                matmuls) + 1 evac/store unit."""

                def __init__(self, c, idx):
                    self.m = 4 * c + idx // 2
                    self.n = idx % 2
                    self.ps = None

                def units(self):
                    def mm():
                        self.ps = ps_work.tile([128, 512], F32, tag="work")
                        for k in range(N_PAIRS):
                            nc.tensor.matmul(
                                self.ps[:],
                                ctxT_sb[:, k, self.m * 128 : (self.m + 1) * 128],
                                wo_sb[:, k, self.n * 512 : (self.n + 1) * 512],
                                start=(k == 0),
                                stop=(k == N_PAIRS - 1),
                            )

                    def evac():
                        ot = out_pool.tile([128, 512], F32, tag="ot")
                        nc.vector.tensor_add(
                            ot[:],
                            self.ps[:],
                            bo_bc[:, self.n * 512 : (self.n + 1) * 512],
                        )
                        nc.sync.dma_start(
                            out_p[
                                self.m * 128 : (self.m + 1) * 128,
                                self.n * 512 : (self.n + 1) * 512,
                            ],
                            ot[:],
                        )

                    return [mm, evac]

            fillers = deque()

            def pump(n, allow_out=True):
                popped = 0
                i = 0
                while popped < n and i < len(fillers):
                    key, kind, emit = fillers[i]
                    if kind == "out" and not allow_out:
                        i += 1
                        continue
                    del fillers[i]
                    emit()
                    popped += 1

            def ensure(key):
                # force-drain the queue (front-first, preserving order) until
                # every unit of `key` has been emitted -- this is what makes
                # the drip schedule correct: consumers only follow producers.
                while any(k == key for k, _, _ in fillers):
                    k, _, emit = fillers.popleft()
                    emit()

            def add_job(key, kind, job):
                for u in job.units():
                    fillers.append((key, kind, u))

            # ---- pre-attention: pair-0 k (chunk 0) + q (chunk 0) ----
            for u in QKJob(1, 0, 0).units():
                u()
            for u in QKJob(0, 0, 0).units():
                u()

            # filler queue in deadline order: v + later k chunks feed pair-0
            # chunk-0's own attention; q c1-3 and all of pair 1 follow.
            # ensure() guarantees correctness; the order here is the prefetch
            # heuristic.
            for lt in range(4):
                add_job(("v", lt), "v", VJob(lt))
            add_job(("k", 0, 1), "qk", QKJob(1, 0, 1))
            for lt in range(4, 8):
                add_job(("v", lt), "v", VJob(lt))
            add_job(("k", 0, 2), "qk", QKJob(1, 0, 2))
            for lt in range(8, 12):
                add_job(("v", lt), "v", VJob(lt))
            add_job(("k", 0, 3), "qk", QKJob(1, 0, 3))
            for lt in range(12, 16):
                add_job(("v", lt), "v", VJob(lt))
            add_job(("q", 0, 1), "qk", QKJob(0, 0, 1))
            add_job(("q", 0, 2), "qk", QKJob(0, 0, 2))
            add_job(("q", 0, 3), "qk", QKJob(0, 0, 3))
            for c in range(NCH):
                add_job(("k", 1, c), "qk", QKJob(1, 1, c))
            for c in range(NCH):
                add_job(("q", 1, c), "qk", QKJob(0, 1, c))

            dbg_done = [False]

            def emit_normalize(p, c, ctx_e, ctx_o):
                # evacuate raw ctx immediately (frees the psum bank), then
                # 1/denom via fast-approx reciprocal, partition-broadcast
                # DMA, and two DVE multiplies into the packed bf16 ctxT.
                raw_e = raw_pool.tile([128, 512], F32, tag="raw")
                raw_o = raw_pool.tile([128, 512], F32, tag="raw")
                nc.vector.tensor_copy(raw_e[:], ctx_e[:])
                nc.vector.tensor_copy(raw_o[:], ctx_o[:])
                if DEBUG and not dbg_done[0]:
                    dbg_done[0] = True
                    nc.sync.dma_start(dbg_raw[:, 0, :], raw_e[:])
                    nc.sync.dma_start(dbg_raw[:, 1, :], raw_o[:])
                rt = recip_pool.tile([128, 512], F32, tag="rt")
                nc.vector.reciprocal(rt[64:65, :], raw_e[64:65, :])
                nc.vector.reciprocal(rt[32:33, :], raw_o[32:33, :])
                # broadcast DMAs ride the scalar queue, which is idle after
                # init (the sync queue carries the out-store traffic).
                bcs = bcs_pool.tile([128, 512], F32, tag="bcs")
                nc.scalar.dma_start(
                    bcs[0:64, :],
                    rt[64:65, :].unsqueeze(1).broadcast_to([1, 64, 512]),
                )
                nc.scalar.dma_start(
                    bcs[64:128, :],
                    rt[32:33, :].unsqueeze(1).broadcast_to([1, 64, 512]),
                )
                sl = slice(c * 512, (c + 1) * 512)
                nc.vector.tensor_mul(
                    ctxT_sb[0:64, p, sl], raw_e[0:64, :], bcs[0:64, :]
                )
                nc.vector.tensor_mul(
                    ctxT_sb[64:128, p, sl], raw_o[64:128, :], bcs[64:128, :]
                )

            # ---- ACT-paced attention: pair-outer, chunk-inner ----
            for p in range(N_PAIRS):
                for c in range(NCH):
                    first_chunk = p == 0 and c == 0
                    ctx_e = ps_acc.tile([128, 512], F32, tag="acc")
                    ctx_o = ps_acc.tile([128, 512], F32, tag="acc")
                    ensure(("q", p, c))
                    et = None
                    for j in range(LT):
                        jp, half = divmod(j, 2)
                        ensure(("k", p, j // 4))
                        if half == 1:
                            ensure(("v", 2 * jp))
                            ensure(("v", 2 * jp + 1))
                        pump(
                            9 if first_chunk else 3,
                            allow_out=(j >= 4),
                        )
                        sT = ps_st.tile([128, 1024], F32, tag="sT")
                        nc.tensor.matmul(
                            sT[:, 0:512],
                            kT_sb[0:64, p, j * 128 : (j + 1) * 128],
                            qT_sb[0:64, p, c * 512 : (c + 1) * 512],
                            start=True,
                            stop=True,
                        )
                        nc.tensor.matmul(
                            sT[:, 512:1024],
                            kT_sb[64:128, p, j * 128 : (j + 1) * 128],
                            qT_sb[64:128, p, c * 512 : (c + 1) * 512],
                            start=True,
                            stop=True,
                        )
                        if half == 0:
                            et = exp_pool.tile([128, 2, 1024], FP8, tag="et")
                        nc.scalar.activation(et[:, half, :], sT[:], Exp, scale=SCALE)
                        if PV8 and half == 1:
                            nc.tensor.matmul(
                                ctx_e[:],
                                vaug[:, p, jp, :, 0:128],
                                et[:, :, 0:512],
                                start=(jp == 0),
                                stop=(jp == NJP - 1),
                                perf_mode=DR,
                            )
                            nc.tensor.matmul(
                                ctx_o[:],
                                vaug[:, p, jp, :, ODD_OFF : ODD_OFF + 128],
                                et[:, :, 512:1024],
                                start=(jp == 0),
                                stop=(jp == NJP - 1),
                                perf_mode=DR,
                            )
                        elif not PV8:
                            nc.tensor.matmul(
                                ctx_e[:],
                                vaug[:, p, jp, half, 0:128],
                                et[:, half, 0:512],
                                start=(j == 0),
                                stop=(j == LT - 1),
                            )
                            nc.tensor.matmul(
                                ctx_o[:],
                                vaug[:, p, jp, half, ODD_OFF : ODD_OFF + 128],
                                et[:, half, 512:1024],
                                start=(j == 0),
                                stop=(j == LT - 1),
                            )
                    emit_normalize(p, c, ctx_e, ctx_o)
                    if p == N_PAIRS - 1:
                        for idx in range(8):
                            add_job(("out", c, idx), "out", OutJob(c, idx))
            pump(len(fillers))
            if DEBUG:
                nc.sync.dma_start(dbg_qT[:], qT_sb[:])
                nc.sync.dma_start(dbg_kT[:], kT_sb[:])
                nc.sync.dma_start(dbg_vaug[:], vaug[:])
                nc.sync.dma_start(dbg_ctxT[:], ctxT_sb[:])
                nc.sync.dma_start(dbg_xT[:], xT_sb[:])
                nc.sync.dma_start(dbg_bc[:, 0:D_LOCAL], bv_bc[:])
                nc.sync.dma_start(dbg_bc[:, D_LOCAL:], bo_bc[:])

    nc.compile()
    return nc


_NC = None
LAST_RESULTS = None


def _get_nc():
    global _NC
    if _NC is None:
        _NC = build_kernel()
    return _NC


def kernel(x, Wq, bq, Wk, bk, Wv, bv, Wo, bo):
    global LAST_RESULTS
    x = np.asarray(x, dtype=np.float32)
    Wq = np.asarray(Wq, dtype=np.float32)
    Wk = np.asarray(Wk, dtype=np.float32)
    Wv = np.asarray(Wv, dtype=np.float32)
    Wo = np.asarray(Wo, dtype=np.float32)
    bq = np.asarray(bq, dtype=np.float32)
    bk = np.asarray(bk, dtype=np.float32)
    bv = np.asarray(bv, dtype=np.float32)
    bo = np.asarray(bo, dtype=np.float32)

    nc = _get_nc()

    xTb = [_to_bf16(x[b].T) for b in range(B)]
    zeros_bo = np.zeros_like(bo)
    in_maps = []
    for c in range(N_CORES):
        b, tp = divmod(c, TP)
        sl = slice(tp * D_LOCAL, (tp + 1) * D_LOCAL)
        in_maps.append(
            {
                "xT": xTb[b],
                "wq": _to_bf16(Wq[:, sl]),
                "wk": _to_bf16(Wk[:, sl]),
                "wv": _to_bf16(Wv[:, sl]),
                "wo": _to_bf16(Wo[sl, :]),
                "bq": np.ascontiguousarray(bq[sl]),
                "bk": np.ascontiguousarray(bk[sl]),
                "bv": np.ascontiguousarray(bv[sl]),
                "bo": np.ascontiguousarray(bo) if tp == 0 else zeros_bo,
            }
        )

    res = run_bass_kernel_spmd(nc, in_maps, core_ids=list(range(N_CORES)))
    LAST_RESULTS = res

    out = np.empty((B, L, D_MODEL), dtype=np.float32)
    for b in range(B):
        acc = res.results[b * TP]["out_p"].astype(np.float32)
        for tp in range(1, TP):
            acc = acc + res.results[b * TP + tp]["out_p"]
        out[b] = acc
    return out


# revision 17
# speedup vs baseline: 1.0251x; 1.0251x over previous
"""Trainium2 Bass kernel for explicit multi-head attention.

Problem: x[2, 2048, 1024], Wq/Wk/Wv/Wo[1024, 1024] (+biases), NHEAD=16.
Sharding over 8 NeuronCores: data-parallel on batch (cores 0-3 -> b=0,
cores 4-7 -> b=1), tensor-parallel over heads (4 heads per core).  Each
core computes its 4 heads' attention plus the partial out-projection
(ctx_local @ Wo[rows_local]); partials are summed on the host, which is
mathematically the all-reduce the sharding hint asks for.

v2 design notes (on top of the v1 transposed-attention layout):
 - Attention starts as early as possible: xT is DMA'd in (k, Lq-chunk)
   pieces, k/q projections for pair-0 chunk-0 are emitted first, and all
   remaining projections (v, later k/q chunks, pair 1, out-proj tiles)
   are drip-fed into the ACT-paced attention loop from a filler queue.
 - PV matmuls run in fp8 DoubleRow perf mode: exp-probs (et) and the
   augmented v tiles are fp8e4; two adjacent key-tiles contract in one
   pass (2x PE throughput).  Softmax weights are quantized but the
   denominator sums the same quantized values, so softmax still sums to
   exactly 1 and the accuracy cost is tiny.
 - Softmax denominators: ctx psum is copied raw to SBUF immediately
   (frees the psum bank in ~1.5us), then reciprocal_approx_fast (~5x
   faster than DVE reciprocal), a partition-broadcast DMA, and two DVE
   multiplies produce the normalized ctxT in bf16.
 - All bias adds ride DVE evacuations (tensor-tensor adds against
   partition-broadcast bias tiles) instead of K=1 rank-1 matmuls.
 - Dummy warm-up matmuls on the first xT piece raise the PE clock gate
   while the input DMAs land.
"""

import os
import sys
from collections import deque

import numpy as np

for _p in ("/opt/trn_rl_repo", "/root/.axon_site/_ro/trn_rl_repo"):
    if os.path.isdir(_p) and _p not in sys.path:
        sys.path.append(_p)

import concourse.bass as bass
import concourse.mybir as mybir
import concourse.tile as tile
from concourse import bacc
from concourse.bass_utils import run_bass_kernel_spmd

# ---------------------------------------------------------------------------
# ACT table-set pinning: this kernel's only activations are Identity and Exp,
# both present in the single "natural_log_exp_and_others" set.  The stock
# per-function chooser maps Exp to "exp_and_others", which alternates
# ACT_TABLE_LOADs (~2.6us each).  Restrict the function->set map so every
# activation resolves to the one set and exactly one table load is emitted.
import concourse.hw_specs as _hw_specs

_orig_get_activation_tables = _hw_specs.get_activation_tables


def _pinned_activation_tables(module_arch):
    t = _orig_get_activation_tables(module_arch)
    pin = {
        mybir.ActivationFunctionType.Exp,
        mybir.ActivationFunctionType.Ln,
        mybir.ActivationFunctionType.Identity,
        mybir.ActivationFunctionType.Copy,
    }
    out = {}
    for name, fns in t.items():
        if name == "natural_log_exp_and_others":
            out[name] = set(fns)
        else:
            out[name] = set(fns) - pin
    return out


bacc.get_activation_tables = _pinned_activation_tables

B = 2
L = 2048
D_MODEL = 1024
NHEAD = 16
D_HEAD = 64
SCALE = 1.0 / float(np.sqrt(D_HEAD))
N_CORES = 8
TP = 4                      # tensor-parallel group size (heads split)
HEADS_PER_CORE = NHEAD // TP          # 4
D_LOCAL = HEADS_PER_CORE * D_HEAD     # 256
N_PAIRS = HEADS_PER_CORE // 2         # 2 head-pairs per core
KT = D_MODEL // 128                   # 8 contraction tiles for projections
LT = L // 128                         # 16 L tiles
NJP = LT // 2                         # 8 key-tile pairs (fp8 DoubleRow)
NCH = L // 512                        # 4 Lq chunks of 512

F32 = mybir.dt.float32
BF16 = mybir.dt.bfloat16
# KERNEL_PV8=0 falls back to bf16 PV matmuls (no fp8 DoubleRow) for debug
PV8 = os.environ.get("KERNEL_PV8", "1") == "1"
FP8 = mybir.dt.float8e4 if PV8 else BF16
PROJ_DT = BF16   # x / Wq / Wk / Wv streamed through the PE
ATT_DT = BF16    # qT / kT score operands
CTX_DT = mybir.dt.float32r  # normalized ctxT (out-proj stationary) and Wo

# v_aug stationary layout, one [128, 2, VAUG_W] fp8 window pair per
# (head-pair, key-tile-pair); dim1 indexes the two key-tiles of the
# DoubleRow contraction.  Window offsets/strides are 16B-aligned to satisfy
# the dual-fp8 LDWEIGHTS ISA restrictions:
#   cols 0:64    = v(even head)          -> even window = cols 0:128
#   col  64      = ones (even denom -> ctx_e partition 64)
#   cols 65:144  = zeros, except
#   col  112     = ones (odd denom; odd window = cols 80:208, so col 112
#                  lands on ctx_o partition 32)
#   cols 144:208 = v(odd head)           -> ctx_o partitions 64:128
VAUG_W = 208
ODD_OFF = 80


def _to_bf16(x):
    import ml_dtypes

    return np.ascontiguousarray(np.asarray(x, np.float32).astype(ml_dtypes.bfloat16))


def build_kernel():
    nc = bacc.Bacc("TRN2", target_bir_lowering=False, debug=False)

    xT = nc.dram_tensor("xT", [D_MODEL, L], PROJ_DT, kind="ExternalInput").ap()
    wq = nc.dram_tensor("wq", [D_MODEL, D_LOCAL], PROJ_DT, kind="ExternalInput").ap()
    wk = nc.dram_tensor("wk", [D_MODEL, D_LOCAL], PROJ_DT, kind="ExternalInput").ap()
    wv = nc.dram_tensor("wv", [D_MODEL, D_LOCAL], PROJ_DT, kind="ExternalInput").ap()
    wo = nc.dram_tensor("wo", [D_LOCAL, D_MODEL], CTX_DT, kind="ExternalInput").ap()
    bq = nc.dram_tensor("bq", [D_LOCAL], F32, kind="ExternalInput").ap()
    bk = nc.dram_tensor("bk", [D_LOCAL], F32, kind="ExternalInput").ap()
    bv = nc.dram_tensor("bv", [D_LOCAL], F32, kind="ExternalInput").ap()
    bo = nc.dram_tensor("bo", [D_MODEL], F32, kind="ExternalInput").ap()
    out_p = nc.dram_tensor("out_p", [L, D_MODEL], F32, kind="ExternalOutput").ap()
    DEBUG = os.environ.get("KERNEL_DEBUG", "0") == "1"
    if DEBUG:
        dbg_qT = nc.dram_tensor("dbg_qT", [128, N_PAIRS, L], ATT_DT, kind="ExternalOutput").ap()
        dbg_kT = nc.dram_tensor("dbg_kT", [128, N_PAIRS, L], ATT_DT, kind="ExternalOutput").ap()
        dbg_vaug = nc.dram_tensor("dbg_vaug", [128, N_PAIRS, NJP, 2, VAUG_W], FP8, kind="ExternalOutput").ap()
        dbg_raw = nc.dram_tensor("dbg_raw", [128, 2, 512], F32, kind="ExternalOutput").ap()
        dbg_ctxT = nc.dram_tensor("dbg_ctxT", [128, N_PAIRS, L], CTX_DT, kind="ExternalOutput").ap()
        dbg_xT = nc.dram_tensor("dbg_xT", [128, KT, L], PROJ_DT, kind="ExternalOutput").ap()
        dbg_bc = nc.dram_tensor("dbg_bc", [128, D_LOCAL + D_MODEL], F32, kind="ExternalOutput").ap()

    Exp = mybir.ActivationFunctionType.Exp
    DR = mybir.MatmulPerfMode.DoubleRow

    with tile.TileContext(nc) as tc:
        with (
            tc.tile_pool(name="persist", bufs=1) as persist,
            tc.tile_pool(name="exp_pool", bufs=3) as exp_pool,
            tc.tile_pool(name="raw_pool", bufs=2) as raw_pool,
            tc.tile_pool(name="recip_pool", bufs=2) as recip_pool,
            tc.tile_pool(name="bcs_pool", bufs=2) as bcs_pool,
            tc.tile_pool(name="out_pool", bufs=4) as out_pool,
            tc.tile_pool(name="ps_st", bufs=2, space="PSUM") as ps_st,
            tc.tile_pool(name="ps_acc", bufs=2, space="PSUM") as ps_acc,
            tc.tile_pool(name="ps_work", bufs=2, space="PSUM") as ps_work,
        ):
            # ---- input DMAs, in dependency-priority order ----
            # sync queue: first xT piece (warm-up dep), wk+wq (pair-0 c0
            # projections), rest of xT chunk 0, wv, then the remaining xT
            # chunks.  scalar queue: small/late tensors.
            xT_sb = persist.tile([128, KT, L], PROJ_DT)

            _xq = [nc.sync, nc.gpsimd, nc.scalar]

            def dma_xT(k, c, nq=3):
                # spread the 4MB xT load over the DMA-capable engine queues
                # (SP/gpsimd/ACT) -- one queue sustains only ~130GB/s and the
                # head is DMA-bound
                _xq[(c * KT + k) % nq].dma_start(
                    xT_sb[:, k, c * 512 : (c + 1) * 512],
                    xT[k * 128 : (k + 1) * 128, c * 512 : (c + 1) * 512],
                )

            dma_xT(0, 0, nq=2)
            wk_sb = persist.tile([128, KT, D_LOCAL], PROJ_DT)
            nc.sync.dma_start(wk_sb[:], wk.rearrange("(k p) n -> p k n", p=128))
            wq_sb = persist.tile([128, KT, D_LOCAL], PROJ_DT)
            nc.sync.dma_start(wq_sb[:], wq.rearrange("(k p) n -> p k n", p=128))
            for k in range(1, KT):
                dma_xT(k, 0, nq=2)
            wv_sb = persist.tile([128, KT, D_LOCAL], PROJ_DT)
            nc.scalar.dma_start(wv_sb[:], wv.rearrange("(k p) n -> p k n", p=128))
            for c in range(1, NCH):
                for k in range(KT):
                    dma_xT(k, c)

            bq_sb = persist.tile([128, D_LOCAL // 128], F32)
            nc.scalar.dma_start(bq_sb[:], bq.rearrange("(m p) -> p m", p=128))
            bk_sb = persist.tile([128, D_LOCAL // 128], F32)
            nc.scalar.dma_start(bk_sb[:], bk.rearrange("(m p) -> p m", p=128))
            bv_ld = persist.tile([1, D_LOCAL], F32)
            nc.scalar.dma_start(bv_ld[:], bv.rearrange("(o n) -> o n", o=1))
            bo_ld = persist.tile([1, D_MODEL], F32)
            nc.scalar.dma_start(bo_ld[:], bo.rearrange("(o n) -> o n", o=1))
            wo_sb = persist.tile([128, N_PAIRS, D_MODEL], CTX_DT)
            nc.scalar.dma_start(wo_sb[:], wo.rearrange("(k p) n -> p k n", p=128))
            # partition-broadcast bias tiles (free-dim biases for v/out)
            bv_bc = persist.tile([128, D_LOCAL], F32)
            nc.scalar.dma_start(
                bv_bc[:], bv_ld[0:1, :].unsqueeze(1).broadcast_to([1, 128, D_LOCAL])
            )
            bo_bc = persist.tile([128, D_MODEL], F32)
            nc.scalar.dma_start(
                bo_bc[:], bo_ld[0:1, :].unsqueeze(1).broadcast_to([1, 128, D_MODEL])
            )

            qT_sb = persist.tile([128, N_PAIRS, L], ATT_DT)
            kT_sb = persist.tile([128, N_PAIRS, L], ATT_DT)
            vaug = persist.tile([128, N_PAIRS, NJP, 2, VAUG_W], FP8)
            ctxT_sb = persist.tile([128, N_PAIRS, L], CTX_DT)

            nc.vector.memset(vaug[:, :, :, :, 64:144], 0.0)
            nc.vector.memset(vaug[:, :, :, :, 64:65], 1.0)
            nc.vector.memset(vaug[:, :, :, :, 112:113], 1.0)

            # ---- PE warm-up: dummy matmuls on the first xT piece while the
            # input DMAs land, so the clock-gate ramp starts immediately ----
            warm = ps_work.tile([128, 512], F32, tag="work")
            for i in range(24):
                nc.tensor.matmul(
                    warm[:, 0:128],
                    xT_sb[0:1, 0, 0:128],
                    xT_sb[0:1, 0, 0:128],
                    start=(i == 0),
                    stop=(i == 23),
                )

            # ---- filler jobs: small emission units dripped into the
            # attention loop to keep the PE dense under the ACT pacer ----
            class QKJob:
                """q or k projection for (tensor t, head-pair m, chunk c):
                4 matmul units (2 chained matmuls each) + 1 bias unit."""

                def __init__(self, t, m, c):
                    self.t, self.m, self.c = t, m, c
                    self.ps = None

                def units(self):
                    w_sb, b_sb, dst = (
                        (wq_sb, bq_sb, qT_sb),
                        (wk_sb, bk_sb, kT_sb),
                    )[self.t]
                    m, c = self.m, self.c

                    def mm(kk):
                        def emit():
                            if kk == 0:
                                self.ps = ps_work.tile([128, 512], F32, tag="work")
                            for k in (2 * kk, 2 * kk + 1):
                                nc.tensor.matmul(
                                    self.ps[:],
                                    w_sb[:, k, m * 128 : (m + 1) * 128],
                                    xT_sb[:, k, c * 512 : (c + 1) * 512],
                                    start=(k == 0),
                                    stop=(k == KT - 1),
                                )

                        return emit

                    def evac():
                        nc.vector.tensor_scalar_add(
                            dst[:, m, c * 512 : (c + 1) * 512],
                            self.ps[:],
                            b_sb[:, m : m + 1],
                        )

                    return [mm(kk) for kk in range(4)] + [evac]

            class VJob:
                """v projection for one key-tile lt: 4 matmul units (2 chained
                matmuls each, single accumulation group -- PSUM start zeroing
                is bank-granular so groups must not share a bank) + 1 evac."""

                def __init__(self, lt):
                    self.lt = lt
                    self.ps = None

                def units(self):
                    lt = self.lt

                    def mm(kk):
                        def emit():
                            if kk == 0:
                                # full-bank tile (cols 256:512 unused) so every
                                # ps_work tenant has an identical footprint
                                self.ps = ps_work.tile([128, 512], F32, tag="work")
                            for k in (2 * kk, 2 * kk + 1):
                                nc.tensor.matmul(
                                    self.ps[:, 0:256],
                                    xT_sb[:, k, lt * 128 : (lt + 1) * 128],
                                    wv_sb[:, k, :],
                                    start=(k == 0),
                                    stop=(k == KT - 1),
                                )

                        return emit

                    def evac():
                        # ps cols: [p0_e p0_o p1_e p1_o]; add bv and scatter
                        # into the fp8 vaug windows.
                        ps_r = self.ps[:, 0:256].rearrange(
                            "p (t q n) -> p t q n", t=2, q=2, n=64
                        )
                        bv_r = bv_bc[:].rearrange(
                            "p (t q n) -> p t q n", t=2, q=2, n=64
                        )
                        jp, sub = divmod(lt, 2)
                        for par, dcol in ((0, 0), (1, 144)):
                            nc.vector.tensor_add(
                                vaug[:, :, jp, sub, dcol : dcol + 64],
                                ps_r[:, :, par, :],
                                bv_r[:, :, par, :],
                            )

                    return [mm(kk) for kk in range(4)] + [evac]

            class OutJob:
                """one [128, 512] tile of out[L, D]: 1 matmul unit (2 chained
                matmuls) + 1 evac/store unit."""

                def __init__(self, c, idx):
                    self.m = 4 * c + idx // 2
                    self.n = idx % 2
                    self.ps = None

                def units(self):
                    def mm():
                        self.ps = ps_work.tile([128, 512], F32, tag="work")
                        for k in range(N_PAIRS):
                            nc.tensor.matmul(
                                self.ps[:],
                                ctxT_sb[:, k, self.m * 128 : (self.m + 1) * 128],
                                wo_sb[:, k, self.n * 512 : (self.n + 1) * 512],
                                start=(k == 0),
                                stop=(k == N_PAIRS - 1),
                            )

                    def evac():
                        ot = out_pool.tile([128, 512], F32, tag="ot")
                        nc.vector.tensor_add(
                            ot[:],
                            self.ps[:],
                            bo_bc[:, self.n * 512 : (self.n + 1) * 512],
                        )
                        nc.sync.dma_start(
                            out_p[
                                self.m * 128 : (self.m + 1) * 128,
                                self.n * 512 : (self.n + 1) * 512,
                            ],
                            ot[:],
                        )

                    return [mm, evac]

            fillers = deque()

            def pump(n, allow_out=True):
                popped = 0
                i = 0
                while popped < n and i < len(fillers):
                    key, kind, emit = fillers[i]
                    if kind == "out" and not allow_out:
                        i += 1
                        continue
                    del fillers[i]
                    emit()
                    popped += 1

            def ensure(key):
                # force-drain the queue (front-first, preserving order) until
                # every unit of `key` has been emitted -- this is what makes
                # the drip schedule correct: consumers only follow producers.
                while any(k == key for k, _, _ in fillers):
                    k, _, emit = fillers.popleft()
                    emit()

            def add_job(key, kind, job):
                for u in job.units():
                    fillers.append((key, kind, u))

            # ---- pre-attention: pair-0 k (chunk 0) + q (chunk 0) ----
            for u in QKJob(1, 0, 0).units():
                u()
            for u in QKJob(0, 0, 0).units():
                u()

            # filler queue in deadline order: v + later k chunks feed pair-0
            # chunk-0's own attention; q c1-3 and all of pair 1 follow.
            # ensure() guarantees correctness; the order here is the prefetch
            # heuristic.
            for lt in range(4):
                add_job(("v", lt), "v", VJob(lt))
            add_job(("k", 0, 1), "qk", QKJob(1, 0, 1))
            for lt in range(4, 8):
                add_job(("v", lt), "v", VJob(lt))
            add_job(("k", 0, 2), "qk", QKJob(1, 0, 2))
            for lt in range(8, 12):
                add_job(("v", lt), "v", VJob(lt))
            add_job(("k", 0, 3), "qk", QKJob(1, 0, 3))
            for lt in range(12, 16):
                add_job(("v", lt), "v", VJob(lt))
            add_job(("q", 0, 1), "qk", QKJob(0, 0, 1))
            add_job(("q", 0, 2), "qk", QKJob(0, 0, 2))
            add_job(("q", 0, 3), "qk", QKJob(0, 0, 3))
            for c in range(NCH):
                add_job(("k", 1, c), "qk", QKJob(1, 1, c))
            for c in range(NCH):
                add_job(("q", 1, c), "qk", QKJob(0, 1, c))

            dbg_done = [False]

            def emit_normalize(p, c, ctx_e, ctx_o):
                # evacuate raw ctx immediately (frees the psum bank), then
                # 1/denom via fast-approx reciprocal, partition-broadcast
                # DMA, and two DVE multiplies into the packed bf16 ctxT.
                raw_e = raw_pool.tile([128, 512], F32, tag="raw")
                raw_o = raw_pool.tile([128, 512], F32, tag="raw")
                nc.vector.tensor_copy(raw_e[:], ctx_e[:])
                nc.vector.tensor_copy(raw_o[:], ctx_o[:])
                if DEBUG and not dbg_done[0]:
                    dbg_done[0] = True
                    nc.sync.dma_start(dbg_raw[:, 0, :], raw_e[:])
                    nc.sync.dma_start(dbg_raw[:, 1, :], raw_o[:])
                # 1/d = exp(-ln d) on the Scalar engine (both functions live
                # in the single pinned ACT table set); keeps the DVE queue
                # free of the 3.4us iterative reciprocal, which otherwise
                # head-of-line-blocks the filler evacs the PE queue waits on.
                rt = recip_pool.tile([128, 512], F32, tag="rt")
                nc.scalar.activation(
                    rt[64:65, :], raw_e[64:65, :], mybir.ActivationFunctionType.Ln
                )
                nc.scalar.activation(
                    rt[64:65, :], rt[64:65, :], Exp, scale=-1.0
                )
                nc.scalar.activation(
                    rt[32:33, :], raw_o[32:33, :], mybir.ActivationFunctionType.Ln
                )
                nc.scalar.activation(
                    rt[32:33, :], rt[32:33, :], Exp, scale=-1.0
                )
                # broadcast DMAs ride the scalar queue, which is idle after
                # init (the sync queue carries the out-store traffic).
                bcs = bcs_pool.tile([128, 512], F32, tag="bcs")
                nc.scalar.dma_start(
                    bcs[0:64, :],
                    rt[64:65, :].unsqueeze(1).broadcast_to([1, 64, 512]),
                )
                nc.scalar.dma_start(
                    bcs[64:128, :],
                    rt[32:33, :].unsqueeze(1).broadcast_to([1, 64, 512]),
                )
                sl = slice(c * 512, (c + 1) * 512)
                nc.vector.tensor_mul(
                    ctxT_sb[0:64, p, sl], raw_e[0:64, :], bcs[0:64, :]
                )
                nc.vector.tensor_mul(
                    ctxT_sb[64:128, p, sl], raw_o[64:128, :], bcs[64:128, :]
                )

            # ---- ACT-paced attention: pair-outer, chunk-inner ----
            for p in range(N_PAIRS):
                for c in range(NCH):
                    first_chunk = p == 0 and c == 0
                    ctx_e = ps_acc.tile([128, 512], F32, tag="acc")
                    ctx_o = ps_acc.tile([128, 512], F32, tag="acc")
                    ensure(("q", p, c))
                    et = None
                    for j in range(LT):
                        jp, half = divmod(j, 2)
                        ensure(("k", p, j // 4))
                        if half == 1:
                            ensure(("v", 2 * jp))
                            ensure(("v", 2 * jp + 1))
                        pump(
                            9 if first_chunk else 3,
                            allow_out=(j >= 2),
                        )
                        sT = ps_st.tile([128, 1024], F32, tag="sT")
                        nc.tensor.matmul(
                            sT[:, 0:512],
                            kT_sb[0:64, p, j * 128 : (j + 1) * 128],
                            qT_sb[0:64, p, c * 512 : (c + 1) * 512],
                            start=True,
                            stop=True,
                        )
                        nc.tensor.matmul(
                            sT[:, 512:1024],
                            kT_sb[64:128, p, j * 128 : (j + 1) * 128],
                            qT_sb[64:128, p, c * 512 : (c + 1) * 512],
                            start=True,
                            stop=True,
                        )
                        if half == 0:
                            et = exp_pool.tile([128, 2, 1024], FP8, tag="et")
                        nc.scalar.activation(et[:, half, :], sT[:], Exp, scale=SCALE)
                        if PV8 and half == 1:
                            nc.tensor.matmul(
                                ctx_e[:],
                                vaug[:, p, jp, :, 0:128],
                                et[:, :, 0:512],
                                start=(jp == 0),
                                stop=(jp == NJP - 1),
                                perf_mode=DR,
                            )
                            nc.tensor.matmul(
                                ctx_o[:],
                                vaug[:, p, jp, :, ODD_OFF : ODD_OFF + 128],
                                et[:, :, 512:1024],
                                start=(jp == 0),
                                stop=(jp == NJP - 1),
                                perf_mode=DR,
                            )
                        elif not PV8:
                            nc.tensor.matmul(
                                ctx_e[:],
                                vaug[:, p, jp, half, 0:128],
                                et[:, half, 0:512],
                                start=(j == 0),
                                stop=(j == LT - 1),
                            )
                            nc.tensor.matmul(
                                ctx_o[:],
                                vaug[:, p, jp, half, ODD_OFF : ODD_OFF + 128],
                                et[:, half, 512:1024],
                                start=(j == 0),
                                stop=(j == LT - 1),
                            )
                    emit_normalize(p, c, ctx_e, ctx_o)
                    if p == N_PAIRS - 1:
                        for idx in range(8):
                            add_job(("out", c, idx), "out", OutJob(c, idx))
            pump(len(fillers))
            if DEBUG:
                nc.sync.dma_start(dbg_qT[:], qT_sb[:])
                nc.sync.dma_start(dbg_kT[:], kT_sb[:])
                nc.sync.dma_start(dbg_vaug[:], vaug[:])
                nc.sync.dma_start(dbg_ctxT[:], ctxT_sb[:])
                nc.sync.dma_start(dbg_xT[:], xT_sb[:])
                nc.sync.dma_start(dbg_bc[:, 0:D_LOCAL], bv_bc[:])
                nc.sync.dma_start(dbg_bc[:, D_LOCAL:], bo_bc[:])

    nc.compile()
    return nc


_NC = None
LAST_RESULTS = None


def _get_nc():
    global _NC
    if _NC is None:
        _NC = build_kernel()
    return _NC


def kernel(x, Wq, bq, Wk, bk, Wv, bv, Wo, bo):
    global LAST_RESULTS
    x = np.asarray(x, dtype=np.float32)
    Wq = np.asarray(Wq, dtype=np.float32)
    Wk = np.asarray(Wk, dtype=np.float32)
    Wv = np.asarray(Wv, dtype=np.float32)
    Wo = np.asarray(Wo, dtype=np.float32)
    bq = np.asarray(bq, dtype=np.float32)
    bk = np.asarray(bk, dtype=np.float32)
    bv = np.asarray(bv, dtype=np.float32)
    bo = np.asarray(bo, dtype=np.float32)

    nc = _get_nc()

    xTb = [_to_bf16(x[b].T) for b in range(B)]
    zeros_bo = np.zeros_like(bo)
    in_maps = []
    for c in range(N_CORES):
        b, tp = divmod(c, TP)
        sl = slice(tp * D_LOCAL, (tp + 1) * D_LOCAL)
        in_maps.append(
            {
                "xT": xTb[b],
                "wq": _to_bf16(Wq[:, sl]),
                "wk": _to_bf16(Wk[:, sl]),
                "wv": _to_bf16(Wv[:, sl]),
                "wo": np.ascontiguousarray(Wo[sl, :], np.float32),
                "bq": np.ascontiguousarray(bq[sl]),
                "bk": np.ascontiguousarray(bk[sl]),
                "bv": np.ascontiguousarray(bv[sl]),
                "bo": np.ascontiguousarray(bo) if tp == 0 else zeros_bo,
            }
        )

    res = run_bass_kernel_spmd(nc, in_maps, core_ids=list(range(N_CORES)))
    LAST_RESULTS = res

    out = np.empty((B, L, D_MODEL), dtype=np.float32)
    for b in range(B):
        acc = res.results[b * TP]["out_p"].astype(np.float32)
        for tp in range(1, TP):
            acc = acc + res.results[b * TP + tp]["out_p"]
        out[b] = acc
    return out
